# revision 11
# baseline (speedup 1.0000x reference)
"""Blockwise soft-quantization (vq_codebook) Trainium2 kernel.

Full inputs in, full outputs out. Internally shards the 1024 blocks
(leading blockified dim) across 8 NeuronCores: core i handles weight rows
[512*i, 512*i+512) = 4 block-rows = 128 blocks, together with the matching
w_min/w_max slices. total_entropy is computed per-core and summed on host.

Self-contained: hardcodes shapes R=C=4096, BLOCK=128, NUM_LEVELS=16, T=100.
"""

import math
import os

import numpy as np

import concourse.bass as bass
import concourse.tile as tile
from concourse import bacc, mybir
from concourse import bass_utils

# ---- problem constants (hardcoded; must match the reference) ----
R = C = 4096
B = 128              # block edge
NLEV = 16            # quantization levels
T = 100.0            # softmax temperature
EPS = 1e-6
NCORES = 8

ROWS_PER_CORE = R // NCORES            # 512
BR_PER_CORE = ROWS_PER_CORE // B       # 4 block-rows
BC = C // B                            # 32 block-cols
NB_CORE = BR_PER_CORE * BC             # 128 blocks per core
G = 4                                  # blocks per column-group
F = G * B                              # 512 free elems per group tile
NGROUPS = BC // G                      # 8 col-groups per block-row

LEVELS = np.linspace(0.0, 1.0, NLEV, dtype=np.float32)

# clamp range for normalized x: mathematically exact for out-of-range x
# (softmax over -T|x - l_k| is invariant in x beyond the end levels), and
# keeps exp(-T*d) of the nearest level >= e^-75 (no Z underflow).
XCLAMP_LO = -0.75
XCLAMP_HI = 1.75

_FP = mybir.dt.float32


def _build_program():
    """Build the per-core Bass/Tile program (SPMD: same NEFF on all cores)."""
    nc = bacc.Bacc("TRN2", target_bir_lowering=False, debug=False)

    w_dram = nc.dram_tensor("weight", [ROWS_PER_CORE, C], _FP, kind="ExternalInput").ap()
    wmin_dram = nc.dram_tensor("w_min", [1, NB_CORE], _FP, kind="ExternalInput").ap()
    wmax_dram = nc.dram_tensor("w_max", [1, NB_CORE], _FP, kind="ExternalInput").ap()
    ident_dram = nc.dram_tensor("identity", [B, B], _FP, kind="ExternalInput").ap()
    ones_dram = nc.dram_tensor("ones", [B, 1], _FP, kind="ExternalInput").ap()
    # activation bias constants, replicated across partitions:
    # cols 0..15: -l_k ; cols 16..31: ln(l_k) (col 16 unused) ; col 32: EPS
    acst_dram = nc.dram_tensor("actconst", [B, 33], _FP, kind="ExternalInput").ap()
    out_dram = nc.dram_tensor("out", [ROWS_PER_CORE, C], _FP, kind="ExternalOutput").ap()
    ent_dram = nc.dram_tensor("ent", [1, 1], _FP, kind="ExternalOutput").ap()

    AF = mybir.ActivationFunctionType
    OP = mybir.AluOpType
    AX = mybir.AxisListType

    with tile.TileContext(nc) as tc:
        with (
            tc.tile_pool(name="const", bufs=1) as const_pool,
            tc.tile_pool(name="io", bufs=3) as io_pool,
            tc.tile_pool(name="xg", bufs=2) as x_pool,
            tc.tile_pool(name="big", bufs=2) as big_pool,
            tc.tile_pool(name="stat", bufs=2) as stat_pool,
            tc.tile_pool(name="ps", bufs=4, space="PSUM") as psum_pool,
        ):
            # ---------------- phase A: per-block scalars ----------------
            ident = const_pool.tile([B, B], _FP)
            nc.sync.dma_start(ident[:], ident_dram)
            ones = const_pool.tile([B, 1], _FP)
            nc.sync.dma_start(ones[:], ones_dram)
            acst = const_pool.tile([B, 33], _FP)
            nc.sync.dma_start(acst[:], acst_dram)

            wm = const_pool.tile([1, NB_CORE], _FP)
            nc.sync.dma_start(wm[:], wmin_dram)
            wx = const_pool.tile([1, NB_CORE], _FP)
            nc.sync.dma_start(wx[:], wmax_dram)

            # wmin_c = min(w_min, w_max - EPS); wmax_c = max(w_max, wmin_c + EPS)
            wmin_c = const_pool.tile([1, NB_CORE], _FP)
            nc.vector.tensor_scalar(wmin_c[:], wx[:], EPS, None, OP.subtract)
            nc.vector.tensor_tensor(wmin_c[:], wm[:], wmin_c[:], OP.min)
            wmax_c = const_pool.tile([1, NB_CORE], _FP)
            nc.vector.tensor_scalar(wmax_c[:], wmin_c[:], EPS, None, OP.add)
            nc.vector.tensor_tensor(wmax_c[:], wx[:], wmax_c[:], OP.max)

            span1 = const_pool.tile([1, NB_CORE], _FP)   # wmax_c - wmin_c
            nc.vector.tensor_tensor(span1[:], wmax_c[:], wmin_c[:], OP.subtract)
            scl1 = const_pool.tile([1, NB_CORE], _FP)    # span + EPS
            nc.vector.tensor_scalar(scl1[:], span1[:], EPS, None, OP.add)
            rscl1 = const_pool.tile([1, NB_CORE], _FP)   # 1/scale
            nc.vector.reciprocal(rscl1[:], scl1[:])
            nwrs1 = const_pool.tile([1, NB_CORE], _FP)   # -wmin*rscale
            nc.vector.tensor_tensor(nwrs1[:], wmin_c[:], rscl1[:], OP.mult)
            nc.vector.tensor_scalar(nwrs1[:], nwrs1[:], -1.0, None, OP.mult)

            # broadcast to all partitions: [128, NB_CORE]
            wmin_bc = const_pool.tile([B, NB_CORE], _FP)
            nc.gpsimd.partition_broadcast(wmin_bc[:], wmin_c[:])
            span_bc = const_pool.tile([B, NB_CORE], _FP)
            nc.gpsimd.partition_broadcast(span_bc[:], span1[:])
            rscl_bc = const_pool.tile([B, NB_CORE], _FP)
            nc.gpsimd.partition_broadcast(rscl_bc[:], rscl1[:])
            nwrs_bc = const_pool.tile([B, NB_CORE], _FP)
            nc.gpsimd.partition_broadcast(nwrs_bc[:], nwrs1[:])

            # accumulator for bin_mass over all core blocks: [c, k, b]
            bin_all = const_pool.tile([B, NLEV, NB_CORE], _FP)

            # ---------------- main loop ----------------
            for br in range(BR_PER_CORE):
                for g in range(NGROUPS):
                    bi0 = br * BC + g * G
                    wrow = io_pool.tile([B, F], _FP, tag="wrow")
                    nc.sync.dma_start(
                        wrow[:], w_dram[br * B:(br + 1) * B, g * F:(g + 1) * F]
                    )

                    # transpose each block; normalize during PSUM->SBUF evac
                    xg = x_pool.tile([B, G, B], _FP, tag="xg")
                    for j in range(G):
                        bi = bi0 + j
                        pt = psum_pool.tile([B, B], _FP, tag="ps")
                        nc.tensor.transpose(pt[:], wrow[:, j * B:(j + 1) * B], ident[:])
                        nc.scalar.activation(
                            xg[:, j, :], pt[:], AF.Identity,
                            bias=nwrs_bc[:, bi:bi + 1], scale=rscl_bc[:, bi:bi + 1],
                        )
                    xf = xg[:].rearrange("p b r -> p (b r)")
                    nc.vector.tensor_scalar(xf, xf, XCLAMP_LO, XCLAMP_HI, OP.max, OP.min)

                    # dists then exp-expansion over 16 levels
                    dq = big_pool.tile([B, NLEV, F], _FP, tag="dq")
                    qs = big_pool.tile([B, NLEV, F], _FP, tag="qs")
                    for k in range(NLEV):
                        nc.scalar.activation(
                            dq[:, k, :], xf, AF.Abs, bias=acst[:, k:k + 1], scale=1.0
                        )
                    # level-weighted numerators: qs_k = l_k * exp(-T d_k)
                    for k in range(1, NLEV):
                        nc.scalar.activation(
                            qs[:, k, :], dq[:, k, :], AF.Exp,
                            bias=acst[:, NLEV + k:NLEV + k + 1], scale=-T,
                        )
                    # q = exp(-T d), one big instruction, in place over dq
                    dqf = dq[:].rearrange("p k f -> p (k f)")
                    nc.scalar.activation(dqf, dqf, AF.Exp, bias=0.0, scale=-T)

                    # Z, 1/Z, Nw, w_q
                    Z = stat_pool.tile([B, 1, F], _FP, tag="Z")
                    nc.vector.reduce_sum(
                        Z[:, 0, :], dq[:].rearrange("p k f -> p f k"), axis=AX.X
                    )
                    zr = stat_pool.tile([B, 1, F], _FP, tag="zr")
                    nc.vector.reciprocal(zr[:, 0, :], Z[:, 0, :])
                    nw = stat_pool.tile([B, F], _FP, tag="nw")
                    nc.vector.reduce_sum(
                        nw[:], qs[:, 1:NLEV, :].rearrange("p k f -> p f k"), axis=AX.X
                    )
                    wq = stat_pool.tile([B, F], _FP, tag="wq")
                    nc.vector.tensor_tensor(wq[:], nw[:], zr[:, 0, :], OP.mult)

                    # p = q * (1/Z)  (in place), then bin_mass accumulation
                    nc.vector.tensor_tensor(
                        dq[:], dq[:], zr[:].to_broadcast((B, NLEV, F)), OP.mult
                    )
                    nc.vector.reduce_sum(
                        bin_all[:, :, bi0:bi0 + G],
                        dq[:].rearrange("p k (b r) -> p k b r", r=B),
                        axis=AX.X,
                    )

                    # transpose back; dequant affine during PSUM->SBUF evac
                    orow = io_pool.tile([B, F], _FP, tag="orow")
                    for j in range(G):
                        bi = bi0 + j
                        pt2 = psum_pool.tile([B, B], _FP, tag="ps")
                        nc.tensor.transpose(pt2[:], wq[:, j * B:(j + 1) * B], ident[:])
                        nc.scalar.activation(
                            orow[:, j * B:(j + 1) * B], pt2[:], AF.Identity,
                            bias=wmin_bc[:, bi:bi + 1], scale=span_bc[:, bi:bi + 1],
                        )
                    nc.sync.dma_start(
                        out_dram[br * B:(br + 1) * B, g * F:(g + 1) * F], orow[:]
                    )

            # ---------------- entropy ----------------
            # column-sum over partitions (c) of bin_all -> [1, NLEV*NB_CORE]
            bflat = bin_all[:].rearrange("p k b -> p (k b)")
            S = const_pool.tile([1, NLEV * NB_CORE], _FP)
            nchunk = (NLEV * NB_CORE) // 512
            for t in range(nchunk):
                cps = psum_pool.tile([1, 512], _FP, tag="ps")
                nc.tensor.matmul(cps[:], ones[:], bflat[:, t * 512:(t + 1) * 512])
                nc.scalar.copy(S[:, t * 512:(t + 1) * 512], cps[:])
            # D_b = sum_k S[k, b] + EPS ; rD = 1/D
            D = const_pool.tile([1, NB_CORE], _FP)
            nc.vector.reduce_sum(
                D[:], S[:].rearrange("p (k b) -> p b k", b=NB_CORE), axis=AX.X
            )
            nc.vector.tensor_scalar(D[:], D[:], EPS, None, OP.add)
            rD = const_pool.tile([1, NB_CORE], _FP)
            nc.vector.reciprocal(rD[:], D[:])
            rD_bc = const_pool.tile([B, 1, NB_CORE], _FP)
            nc.gpsimd.partition_broadcast(rD_bc[:].rearrange("p a b -> p (a b)"), rD[:])
            # bin_probs (in place over bin_all), log, entropy accumulate
            bp = bin_all
            nc.vector.tensor_tensor(
                bp[:], bp[:], rD_bc[:].to_broadcast((B, NLEV, NB_CORE)), OP.mult
            )
            lg = const_pool.tile([B, NLEV, NB_CORE], _FP)
            nc.scalar.activation(lg[:], bp[:], AF.Ln, bias=acst[:, 32:33], scale=1.0)
            nc.vector.tensor_tensor(bp[:], bp[:], lg[:], OP.mult)
            entv = const_pool.tile([B, 1], _FP)
            nc.vector.reduce_sum(
                entv[:], bp[:].rearrange("p k b -> p (k b)"), axis=AX.X
            )
            eps_ = psum_pool.tile([1, 1], _FP, tag="ps")
            nc.tensor.matmul(eps_[:], ones[:], entv[:])
            esb = const_pool.tile([1, 1], _FP)
            nc.scalar.mul(esb[:], eps_[:], -1.0)
            nc.sync.dma_start(ent_dram, esb[:])

    nc.compile()
    return nc


def _actconst():
    row = np.zeros(33, dtype=np.float32)
    row[0:NLEV] = -LEVELS
    row[NLEV + 1:2 * NLEV] = np.log(LEVELS[1:].astype(np.float64)).astype(np.float32)
    row[32] = EPS
    return np.tile(row[None, :], (B, 1))


_CACHED_NC = None


def _get_nc():
    global _CACHED_NC
    if _CACHED_NC is None:
        _CACHED_NC = _build_program()
    return _CACHED_NC


LAST_RESULTS = None  # stashed BassKernelResults (for test harness inspection)


def kernel(weight, w_min, w_max):
    global LAST_RESULTS
    weight = np.asarray(weight, dtype=np.float32)
    w_min = np.asarray(w_min, dtype=np.float32)
    w_max = np.asarray(w_max, dtype=np.float32)
    assert weight.shape == (R, C) and w_min.shape == (R // B * C // B,)

    ident = np.eye(B, dtype=np.float32)
    ones = np.ones((B, 1), dtype=np.float32)
    acst = _actconst()

    in_maps = []
    for cid in range(NCORES):
        in_maps.append({
            "weight": np.ascontiguousarray(
                weight[cid * ROWS_PER_CORE:(cid + 1) * ROWS_PER_CORE]
            ),
            "w_min": np.ascontiguousarray(
                w_min[cid * NB_CORE:(cid + 1) * NB_CORE].reshape(1, NB_CORE)
            ),
            "w_max": np.ascontiguousarray(
                w_max[cid * NB_CORE:(cid + 1) * NB_CORE].reshape(1, NB_CORE)
            ),
            "identity": ident,
            "ones": ones,
            "actconst": acst,
        })

    nc = _get_nc()
    res = bass_utils.run_bass_kernel_spmd(
        nc, in_maps, core_ids=list(range(NCORES)), trace=False
    )
    LAST_RESULTS = res

    out = np.empty((R, C), dtype=np.float32)
    ent = np.float32(0.0)
    for cid in range(NCORES):
        out[cid * ROWS_PER_CORE:(cid + 1) * ROWS_PER_CORE] = res.results[cid]["out"]
        ent = np.float32(ent + res.results[cid]["ent"][0, 0])
    return out, ent


# revision 19
# speedup vs baseline: 1.0783x; 1.0783x over previous
"""Blockwise soft-quantization (vq_codebook) Trainium2 kernel.

Full inputs in, full outputs out. Internally shards the 1024 blocks
(leading blockified dim) across 8 NeuronCores: core i handles weight rows
[512*i, 512*i+512) = 4 block-rows = 128 blocks, together with the matching
w_min/w_max slices. total_entropy is computed per-core and summed on host.

Self-contained: hardcodes shapes R=C=4096, BLOCK=128, NUM_LEVELS=16, T=100.
"""

import math
import os

import numpy as np

import concourse.bass as bass
import concourse.tile as tile
from concourse import bacc, mybir
from concourse import bass_utils

# ---- problem constants (hardcoded; must match the reference) ----
R = C = 4096
B = 128              # block edge
NLEV = 16            # quantization levels
T = 100.0            # softmax temperature
EPS = 1e-6
NCORES = 8

ROWS_PER_CORE = R // NCORES            # 512
BR_PER_CORE = ROWS_PER_CORE // B       # 4 block-rows
BC = C // B                            # 32 block-cols
NB_CORE = BR_PER_CORE * BC             # 128 blocks per core
G = 4                                  # blocks per column-group
F = G * B                              # 512 free elems per group tile
NGROUPS = BC // G                      # 8 col-groups per block-row

LEVELS = np.linspace(0.0, 1.0, NLEV, dtype=np.float32)

# clamp range for normalized x: mathematically exact for out-of-range x
# (softmax over -T|x - l_k| is invariant in x beyond the end levels), and
# keeps exp(-T*d) of the nearest level >= e^-75 (no Z underflow).
XCLAMP_LO = -0.75
XCLAMP_HI = 1.75

_FP = mybir.dt.float32


VARIANT = {
    "qs": True,       # level-weighted numerator pass (ACT) + Nw reduce
    "pmul": True,     # p = q*zr (GPSIMD) 
    "bin": True,      # bin_mass reduce (DVE)
    "absexp": True,   # the 16-level Abs+Exp expansion
    "zred": True,     # Z reduce
    "pmul_engine": "gpsimd",
}


def _build_program():
    """Build the per-core Bass/Tile program (SPMD: same NEFF on all cores)."""
    V = VARIANT
    nc = bacc.Bacc("TRN2", target_bir_lowering=False, debug=False)

    w_dram = nc.dram_tensor("weight", [ROWS_PER_CORE, C], _FP, kind="ExternalInput").ap()
    wmin_dram = nc.dram_tensor("w_min", [1, NB_CORE], _FP, kind="ExternalInput").ap()
    wmax_dram = nc.dram_tensor("w_max", [1, NB_CORE], _FP, kind="ExternalInput").ap()
    ident_dram = nc.dram_tensor("identity", [B, B], _FP, kind="ExternalInput").ap()
    ones_dram = nc.dram_tensor("ones", [B, 1], _FP, kind="ExternalInput").ap()
    # activation bias constants, replicated across partitions:
    # cols 0..15: -l_k ; cols 16..31: ln(l_k) (col 16 unused) ; col 32: EPS
    acst_dram = nc.dram_tensor("actconst", [B, 33], _FP, kind="ExternalInput").ap()
    out_dram = nc.dram_tensor("out", [ROWS_PER_CORE, C], _FP, kind="ExternalOutput").ap()
    ent_dram = nc.dram_tensor("ent", [1, 1], _FP, kind="ExternalOutput").ap()

    AF = mybir.ActivationFunctionType
    OP = mybir.AluOpType
    AX = mybir.AxisListType

    with tile.TileContext(nc) as tc:
        with (
            tc.tile_pool(name="const", bufs=1) as const_pool,
            tc.tile_pool(name="io", bufs=4) as io_pool,
            tc.tile_pool(name="xg", bufs=3) as x_pool,
            tc.tile_pool(name="big", bufs=3) as big_pool,
            tc.tile_pool(name="qsp", bufs=1) as qs_pool,
            tc.tile_pool(name="stat", bufs=3) as stat_pool,
            tc.tile_pool(name="ps", bufs=8, space="PSUM") as psum_pool,
        ):
            # ---------------- phase A: per-block scalars ----------------
            ident = const_pool.tile([B, B], _FP)
            nc.sync.dma_start(ident[:], ident_dram)
            ones = const_pool.tile([B, 1], _FP)
            nc.sync.dma_start(ones[:], ones_dram)
            acst = const_pool.tile([B, 33], _FP)
            nc.sync.dma_start(acst[:], acst_dram)

            wm = const_pool.tile([1, NB_CORE], _FP)
            nc.sync.dma_start(wm[:], wmin_dram)
            wx = const_pool.tile([1, NB_CORE], _FP)
            nc.sync.dma_start(wx[:], wmax_dram)

            # wmin_c = min(w_min, w_max - EPS); wmax_c = max(w_max, wmin_c + EPS)
            wmin_c = const_pool.tile([1, NB_CORE], _FP)
            nc.vector.tensor_scalar(wmin_c[:], wx[:], EPS, None, OP.subtract)
            nc.vector.tensor_tensor(wmin_c[:], wm[:], wmin_c[:], OP.min)
            wmax_c = const_pool.tile([1, NB_CORE], _FP)
            nc.vector.tensor_scalar(wmax_c[:], wmin_c[:], EPS, None, OP.add)
            nc.vector.tensor_tensor(wmax_c[:], wx[:], wmax_c[:], OP.max)

            span1 = const_pool.tile([1, NB_CORE], _FP)   # wmax_c - wmin_c
            nc.vector.tensor_tensor(span1[:], wmax_c[:], wmin_c[:], OP.subtract)
            scl1 = const_pool.tile([1, NB_CORE], _FP)    # span + EPS
            nc.vector.tensor_scalar(scl1[:], span1[:], EPS, None, OP.add)
            rscl1 = const_pool.tile([1, NB_CORE], _FP)   # 1/scale
            nc.vector.reciprocal(rscl1[:], scl1[:])
            nwrs1 = const_pool.tile([1, NB_CORE], _FP)   # -wmin*rscale
            nc.vector.tensor_tensor(nwrs1[:], wmin_c[:], rscl1[:], OP.mult)
            nc.vector.tensor_scalar(nwrs1[:], nwrs1[:], -1.0, None, OP.mult)

            # broadcast to all partitions: [128, NB_CORE]
            wmin_bc = const_pool.tile([B, NB_CORE], _FP)
            nc.gpsimd.partition_broadcast(wmin_bc[:], wmin_c[:])
            span_bc = const_pool.tile([B, NB_CORE], _FP)
            nc.gpsimd.partition_broadcast(span_bc[:], span1[:])
            rscl_bc = const_pool.tile([B, NB_CORE], _FP)
            nc.gpsimd.partition_broadcast(rscl_bc[:], rscl1[:])
            nwrs_bc = const_pool.tile([B, NB_CORE], _FP)
            nc.gpsimd.partition_broadcast(nwrs_bc[:], nwrs1[:])

            # accumulator for bin_mass over all core blocks: [c, k, b]
            bin_all = const_pool.tile([B, NLEV, NB_CORE], _FP)

            # ---------------- main loop: 3-stage software pipeline ----------
            # stage A(t): dma-in, PE transpose, ACT evac+normalize, DVE clamp
            # stage B(t): ACT abs/qs/exp, DVE Z/recip/Nw/wq, GPSIMD p
            # stage C(t): DVE bin-reduce, PE out-transpose, ACT dequant evac,
            #             dma-out
            # Emitting A(t+1) before B(t) before C(t-1) keeps every in-order
            # engine supplied with ready work (no DVE<->ACT ping-pong stalls).
            NG = BR_PER_CORE * NGROUPS
            st = {}

            def stage_a(t):
                br, g = divmod(t, NGROUPS)
                bi0 = br * BC + g * G
                wrow = io_pool.tile([B, F], _FP, tag="wrow")
                nc.sync.dma_start(
                    wrow[:], w_dram[br * B:(br + 1) * B, g * F:(g + 1) * F]
                )
                xg = x_pool.tile([B, G, B], _FP, tag="xg")
                for j in range(G):
                    bi = bi0 + j
                    pt = psum_pool.tile([B, B], _FP, tag="ps")
                    nc.tensor.transpose(pt[:], wrow[:, j * B:(j + 1) * B], ident[:])
                    nc.scalar.activation(
                        xg[:, j, :], pt[:], AF.Identity,
                        bias=nwrs_bc[:, bi:bi + 1], scale=rscl_bc[:, bi:bi + 1],
                    )
                xf = xg[:].rearrange("p b r -> p (b r)")
                nc.vector.tensor_scalar(xf, xf, XCLAMP_LO, XCLAMP_HI, OP.max, OP.min)
                st[t] = {"xf": xf, "bi0": bi0, "br": br, "g": g}

            def stage_b(t):
                s = st[t]
                dq = big_pool.tile([B, NLEV, F], _FP, tag="dq")
                qs = qs_pool.tile([B, NLEV - 1, F], _FP, tag="qs")
                if V["absexp"]:
                    for k in range(NLEV):
                        nc.scalar.activation(
                            dq[:, k, :], s["xf"], AF.Abs,
                            bias=acst[:, k:k + 1], scale=1.0,
                        )
                if V["qs"]:
                    for k in range(1, NLEV):
                        nc.scalar.activation(
                            qs[:, k - 1, :], dq[:, k, :], AF.Exp,
                            bias=acst[:, NLEV + k:NLEV + k + 1], scale=-T,
                        )
                if V["absexp"]:
                    dqf = dq[:].rearrange("p k f -> p (k f)")
                    nc.scalar.activation(dqf, dqf, AF.Exp, bias=0.0, scale=-T)

                Z = stat_pool.tile([B, 1, F], _FP, tag="Z")
                if V["zred"]:
                    nc.vector.reduce_sum(
                        Z[:, 0, :], dq[:].rearrange("p k f -> p f k"), axis=AX.X
                    )
                else:
                    nc.vector.tensor_scalar(Z[:, 0, :], dq[:, 0, :], 1.0, None, OP.add)
                zr = Z
                nc.vector.reciprocal(zr[:, 0, :], Z[:, 0, :])
                nw = stat_pool.tile([B, F], _FP, tag="nw")
                if V["qs"]:
                    nc.vector.reduce_sum(
                        nw[:], qs[:].rearrange("p k f -> p f k"), axis=AX.X
                    )
                else:
                    nc.vector.tensor_scalar(nw[:], dq[:, 0, :], 1.0, None, OP.add)
                nc.vector.tensor_tensor(nw[:], nw[:], zr[:, 0, :], OP.mult)
                if V["pmul"]:
                    eng = nc.gpsimd if V["pmul_engine"] == "gpsimd" else nc.vector
                    eng.tensor_tensor(
                        dq[:], dq[:], zr[:].to_broadcast((B, NLEV, F)), OP.mult
                    )
                s["dq"] = dq
                s["wq"] = nw

            def stage_c(t):
                s = st.pop(t)
                bi0, br, g = s["bi0"], s["br"], s["g"]
                if V["bin"]:
                    nc.vector.reduce_sum(
                        bin_all[:, :, bi0:bi0 + G],
                        s["dq"][:].rearrange("p k (b r) -> p k b r", r=B),
                        axis=AX.X,
                    )
                wq = s["wq"]
                orow = io_pool.tile([B, F], _FP, tag="orow")
                for j in range(G):
                    bi = bi0 + j
                    pt2 = psum_pool.tile([B, B], _FP, tag="ps")
                    nc.tensor.transpose(pt2[:], wq[:, j * B:(j + 1) * B], ident[:])
                    nc.scalar.activation(
                        orow[:, j * B:(j + 1) * B], pt2[:], AF.Identity,
                        bias=wmin_bc[:, bi:bi + 1], scale=span_bc[:, bi:bi + 1],
                    )
                nc.sync.dma_start(
                    out_dram[br * B:(br + 1) * B, g * F:(g + 1) * F], orow[:]
                )

            for t in range(NG + 2):
                if t < NG:
                    stage_a(t)
                if 1 <= t <= NG:
                    stage_b(t - 1)
                if t >= 2:
                    stage_c(t - 2)

            # ---------------- entropy ----------------
            # column-sum over partitions (c) of bin_all -> [1, NLEV*NB_CORE]
            bflat = bin_all[:].rearrange("p k b -> p (k b)")
            S = const_pool.tile([1, NLEV * NB_CORE], _FP)
            nchunk = (NLEV * NB_CORE) // 512
            for t in range(nchunk):
                cps = psum_pool.tile([1, 512], _FP, tag="ps")
                nc.tensor.matmul(cps[:], ones[:], bflat[:, t * 512:(t + 1) * 512])
                nc.scalar.copy(S[:, t * 512:(t + 1) * 512], cps[:])
            # D_b = sum_k S[k, b] + EPS ; rD = 1/D
            D = const_pool.tile([1, NB_CORE], _FP)
            nc.vector.reduce_sum(
                D[:], S[:].rearrange("p (k b) -> p b k", b=NB_CORE), axis=AX.X
            )
            nc.vector.tensor_scalar(D[:], D[:], EPS, None, OP.add)
            rD = const_pool.tile([1, NB_CORE], _FP)
            nc.vector.reciprocal(rD[:], D[:])
            rD_bc = const_pool.tile([B, 1, NB_CORE], _FP)
            nc.gpsimd.partition_broadcast(rD_bc[:].rearrange("p a b -> p (a b)"), rD[:])
            # bin_probs (in place over bin_all), log, entropy accumulate
            bp = bin_all
            nc.vector.tensor_tensor(
                bp[:], bp[:], rD_bc[:].to_broadcast((B, NLEV, NB_CORE)), OP.mult
            )
            lg = const_pool.tile([B, NLEV, NB_CORE], _FP)
            nc.scalar.activation(lg[:], bp[:], AF.Ln, bias=acst[:, 32:33], scale=1.0)
            nc.vector.tensor_tensor(bp[:], bp[:], lg[:], OP.mult)
            entv = const_pool.tile([B, 1], _FP)
            nc.vector.reduce_sum(
                entv[:], bp[:].rearrange("p k b -> p (k b)"), axis=AX.X
            )
            eps_ = psum_pool.tile([1, 1], _FP, tag="ps")
            nc.tensor.matmul(eps_[:], ones[:], entv[:])
            esb = const_pool.tile([1, 1], _FP)
            nc.scalar.mul(esb[:], eps_[:], -1.0)
            nc.sync.dma_start(ent_dram, esb[:])

    nc.compile()
    return nc


def _actconst():
    row = np.zeros(33, dtype=np.float32)
    row[0:NLEV] = -LEVELS
    row[NLEV + 1:2 * NLEV] = np.log(LEVELS[1:].astype(np.float64)).astype(np.float32)
    row[32] = EPS
    return np.tile(row[None, :], (B, 1))


_CACHED_NC = None


def _get_nc():
    global _CACHED_NC
    if _CACHED_NC is None:
        _CACHED_NC = _build_program()
    return _CACHED_NC


LAST_RESULTS = None  # stashed BassKernelResults (for test harness inspection)


def kernel(weight, w_min, w_max):
    global LAST_RESULTS
    weight = np.asarray(weight, dtype=np.float32)
    w_min = np.asarray(w_min, dtype=np.float32)
    w_max = np.asarray(w_max, dtype=np.float32)
    assert weight.shape == (R, C) and w_min.shape == (R // B * C // B,)

    ident = np.eye(B, dtype=np.float32)
    ones = np.ones((B, 1), dtype=np.float32)
    acst = _actconst()

    in_maps = []
    for cid in range(NCORES):
        in_maps.append({
            "weight": np.ascontiguousarray(
                weight[cid * ROWS_PER_CORE:(cid + 1) * ROWS_PER_CORE]
            ),
            "w_min": np.ascontiguousarray(
                w_min[cid * NB_CORE:(cid + 1) * NB_CORE].reshape(1, NB_CORE)
            ),
            "w_max": np.ascontiguousarray(
                w_max[cid * NB_CORE:(cid + 1) * NB_CORE].reshape(1, NB_CORE)
            ),
            "identity": ident,
            "ones": ones,
            "actconst": acst,
        })

    nc = _get_nc()
    res = bass_utils.run_bass_kernel_spmd(
        nc, in_maps, core_ids=list(range(NCORES)), trace=False
    )
    LAST_RESULTS = res

    out = np.empty((R, C), dtype=np.float32)
    ent = np.float32(0.0)
    for cid in range(NCORES):
        out[cid * ROWS_PER_CORE:(cid + 1) * ROWS_PER_CORE] = res.results[cid]["out"]
        ent = np.float32(ent + res.results[cid]["ent"][0, 0])
    return out, ent
